# revision 41
# baseline (speedup 1.0000x reference)
"""Trainium2 Bass kernel for nn_MixedLoraModel_734 (v2).

Computes, for T=8192 tokens, D=O=4096:
    out = x @ W_base^T + b_base + scaling[token_lora][:,None] * lora(x)
where lora(x)[t] = WB[l_t] @ (WA[l_t] @ x[t]),  l_t = token_lora[t],
L=8 adapters of rank R=16 (adapter stack = 128 rows).

Strategy (8 NeuronCores, data-parallel over tokens), v2:
  - Weights are prepped on host (offline weight transform, as a serving
    stack would do): W_base^T is stored bf16 in oc-blocked layout
    wtb[33, 128, 32*128] (block 0 = WA_flat^T for the LoRA `u` GEMM,
    blocks 1..32 = 128-wide o-chunks of W^T), wbsT = bf16
    (scaling * WB)^T in [(l,r), o] layout, bias in [128, 32] blocked
    form, jdiv = arange(128)//16 for the routing mask.
  - On device the PE does *only* matmuls: per o-chunk, 32 stationary
    [d,o] tiles stream from DRAM (plain contiguous DMA, no transposes,
    no casts) against a resident bf16 x^T; one extra matmul per chunk
    adds the routed LoRA term; the u-GEMM is just block 0 of the same
    pipeline.
  - x is cast to bf16 on host; its ingestion is 8 DMA-XBAR transposes
    straight from DRAM into resident xT2, in [p, (tt, dc, t)] layout so
    every XBAR destination is contiguous.  All XBAR transposes ride the
    sync queue only (concurrent XBAR transposes on both HW queues
    corrupt each other).
  - Routing is dense: maskT[j, t] = (token_lora[t] == j//16) built from
    a PE broadcast + DVE compare; u_m = mask * u evicted to bf16.
  - Output path: scalar eviction (bias add, bf16 cast) -> DMA-XBAR
    transpose back to natural [t, o] -> DVE cast to f32 -> store via
    the gpsimd software-DGE queue (keeps HW queues free for x/W).
  - Token halves are software-pipelined (h0 runs `LAG` blocks ahead of
    h1) so the PE can start on the first half of x while the second
    half is still streaming in.
"""

import numpy as np

import concourse.bass as bass
import concourse.mybir as mybir
import concourse.tile as tile
from concourse import bacc

P = 128
D = 4096          # d_in
O = 4096          # d_out
NCORES = 8
T = 8192
TS = T // NCORES  # 1024 tokens per core
NT = TS // P      # 8 token tiles per core
ND = D // P       # 32 contraction chunks
NOC = O // P      # 32 output chunks of 128
HT = TS // 2      # 512-wide halves
NB = NOC + 1      # u block + 32 o-chunks
L, R, LR = 8, 16, 128
LAG = 4           # h0 runs LAG blocks ahead of h1

F32 = mybir.dt.float32
BF16 = mybir.dt.bfloat16
I32 = mybir.dt.int32
EQ = mybir.AluOpType.is_equal
MUL = mybir.AluOpType.mult
IDENT = mybir.ActivationFunctionType.Identity


def _build(debug: bool = False) -> bass.Bass:
    nc = bacc.Bacc(None)

    xt = nc.declare_dram_parameter("xt", [P, NT * ND * P], BF16,
                                   isOutput=False)
    tl = nc.declare_dram_parameter("tl", [TS], I32, isOutput=False)
    wtb = nc.declare_dram_parameter("wtb", [NB, P, ND * P], BF16,
                                    isOutput=False)
    wbst = nc.declare_dram_parameter("wbst", [P, O], BF16, isOutput=False)
    cvec = nc.declare_dram_parameter("cvec", [P, 1 + NOC], F32,
                                     isOutput=False)
    out = nc.declare_dram_parameter("out", [TS, O], F32, isOutput=True)
    if debug:
        xt_dump = nc.declare_dram_parameter("xt_dump", [P, NT * ND * P],
                                            BF16, isOutput=True)
        um_dump = nc.declare_dram_parameter("um_dump", [P, TS], BF16,
                                            isOutput=True)
        mask_dump = nc.declare_dram_parameter("mask_dump", [P, TS], F32,
                                              isOutput=True)

    with tile.TileContext(nc) as tc:
        with (
            tc.tile_pool(name="const", bufs=1) as const,
            tc.tile_pool(name="res", bufs=1) as res,
            tc.tile_pool(name="wt", bufs=7) as wt_p,
            tc.tile_pool(name="outT", bufs=3) as outT_p,
            tc.tile_pool(name="onat", bufs=3) as onat_p,
            tc.tile_pool(name="outf", bufs=3) as outf_p,
        ):
            # ---- resident tiles ----
            # xT2 layout: [p, (h, dc, t)] — x^T in half-token-major,
            # contraction-chunk blocks; each (h, dc) slab is a contiguous
            # 512-wide moving operand.
            xT2 = res.tile([P, NT * ND * P], BF16, tag="xT2")
            wbsT = res.tile([P, O], BF16, tag="wbsT")
            cvec_sb = const.tile([P, 1 + NOC], F32)
            jdiv_sb = cvec_sb[:, 0:1]
            bb_sb = cvec_sb[:, 1:1 + NOC]
            u_m = res.tile([P, TS], BF16, tag="u_m")
            maskT = res.tile([P, TS], F32, tag="maskT")
            ones_row = const.tile([1, P], F32)
            tli = const.tile([1, TS], I32)
            tlf = const.tile([1, TS], F32)

            # ---- emission helpers ----
            wt_tiles = {}

            def w_load(b, q):
                wt = wt_p.tile([P, ND * P], BF16, tag="wt", name=f"wt{b}")
                q.dma_start(out=wt[:], in_=wtb[b, :, :])
                wt_tiles[b] = wt

            # ---- prepass emission ----
            # x^T arrives from the host already in the xT2 SBUF image, so
            # ingestion is 4 plain contiguous DMAs split across both HW
            # queues (quarter-token-halves; the tile scheduler chains big
            # DMAs with false serialization edges, so keep them few and
            # balanced).  XBAR transposes are reserved for the out path
            # (sync queue only: two concurrent XBAR transposes on
            # different queues corrupt each other).
            # Three load channels (sync HWDGE, scalar HWDGE, gpsimd
            # SWDGE), h0-critical bytes first on each.  Per-queue DMAs
            # run one-at-a-time (~95-150 GB/s each), so balance by bytes.
            XCW = NT * ND * P // 8  # x chunk width (8 chunks, asc = h0 1st)

            def x_chunk(k, q):
                q.dma_start(out=xT2[:, k * XCW:(k + 1) * XCW],
                            in_=xt[:, k * XCW:(k + 1) * XCW])

            # W0 split in halves so the very first u matmuls unblock early
            wt0 = wt_p.tile([P, ND * P], BF16, tag="wt", name="wt0")
            wt_tiles[0] = wt0
            HW = ND * P // 2
            nc.gpsimd.dma_start(out=wt0[:, 0:HW], in_=wtb[0, :, 0:HW])
            nc.gpsimd.dma_start(out=wt0[:, HW:2 * HW],
                                in_=wtb[0, :, HW:2 * HW])
            x_chunk(0, nc.sync)
            x_chunk(1, nc.scalar)
            nc.gpsimd.dma_start(out=tli[:],
                                in_=tl.rearrange("(a f) -> a f", a=1))
            nc.gpsimd.dma_start(out=cvec_sb[:], in_=cvec[:, :])
            x_chunk(2, nc.gpsimd)
            x_chunk(3, nc.sync)
            x_chunk(4, nc.scalar)
            w_load(1, nc.gpsimd)
            x_chunk(6, nc.sync)
            x_chunk(7, nc.scalar)
            nc.gpsimd.dma_start(out=wbsT[:], in_=wbst[:, :])
            x_chunk(5, nc.gpsimd)
            w_load(2, nc.sync)
            w_load(3, nc.scalar)
            w_load(4, nc.gpsimd)
            nc.vector.memset(ones_row[:], 1.0)
            nc.vector.tensor_copy(tlf[:], tli[:])

            def emit_masks():
                # routing mask: broadcast token ids across partitions
                # (PE), compare against j//16 (DVE).  Emitted AFTER the
                # first half-iter's matmuls so the in-order PE stream is
                # not head-blocked waiting for the token-id row.
                with tc.tile_pool(name="psM", bufs=2, space="PSUM") as psM:
                    for h in range(2):
                        tlbc = psM.tile([P, HT], F32, tag="tlbc",
                                        name=f"tlbc{h}")
                        nc.tensor.matmul(tlbc[:], ones_row[:],
                                         tlf[0:1, h * HT:(h + 1) * HT],
                                         start=True, stop=True)
                        nc.vector.tensor_scalar(
                            maskT[:, h * HT:(h + 1) * HT],
                            tlbc[:], jdiv_sb, None, EQ)

            # ---- main loop: software-pipelined halves ----
            with (
                tc.tile_pool(name="psU", bufs=2, space="PSUM") as psU,
                tc.tile_pool(name="psA", bufs=4, space="PSUM") as psA,
            ):
                u_ps = {}
                next_w = 5

                def u_mms(h):
                    acc = psU.tile([P, HT], F32, tag="ups", name=f"ups{h}")
                    u_ps[h] = acc
                    wt = wt_tiles[0]
                    for dc in range(ND):
                        mov = xT2[:, (h * ND + dc) * HT:
                                  (h * ND + dc + 1) * HT]
                        nc.tensor.matmul(acc[:],
                                         wt[:, dc * P:(dc + 1) * P], mov,
                                         start=(dc == 0),
                                         stop=(dc == ND - 1))

                def u_evict(h):
                    # mask + bf16 eviction of the selected u
                    nc.vector.tensor_tensor(
                        u_m[:, h * HT:(h + 1) * HT], u_ps[h][:],
                        maskT[:, h * HT:(h + 1) * HT], MUL)

                def half_iter(b, h):
                    nonlocal next_w
                    if h == 0 and next_w < NB:
                        w_load(next_w, nc.sync if next_w % 2 else nc.scalar)
                        next_w += 1
                    wt = wt_tiles[b]
                    acc = psA.tile([P, HT], F32, tag="acc",
                                   name=f"acc{b}_{h}")
                    for dc in range(ND):
                        mov = xT2[:, (h * ND + dc) * HT:
                                  (h * ND + dc + 1) * HT]
                        nc.tensor.matmul(acc[:],
                                         wt[:, dc * P:(dc + 1) * P], mov,
                                         start=(dc == 0), stop=False)
                    oc = b - 1
                    nc.tensor.matmul(acc[:], wbsT[:, oc * P:(oc + 1) * P],
                                     u_m[:, h * HT:(h + 1) * HT],
                                     start=False, stop=True)
                    # evict: bias add + bf16 cast on scalar engine
                    oT = outT_p.tile([P, HT], BF16, tag="oT",
                                     name=f"oT{oc}_{h}")
                    nc.scalar.activation(oT[:], acc[:], IDENT,
                                         bias=cvec_sb[:, 1 + oc:2 + oc])
                    # XBAR transpose back to natural token-major layout
                    onat = onat_p.tile([P, HT], BF16, tag="onat",
                                       name=f"onat{oc}_{h}")
                    # sync queue only: XBAR transposes must never overlap
                    nc.sync.dma_start_transpose(
                        onat[:].rearrange("p (k o) -> p k o", o=P), oT[:])
                    # f32 cast + store via gpsimd software DGE
                    of = outf_p.tile([P, HT], F32, tag="of",
                                     name=f"of{oc}_{h}")
                    nc.vector.tensor_copy(of[:], onat[:])
                    dst = out[h * HT:(h + 1) * HT, oc * P:(oc + 1) * P] \
                        .rearrange("(k t) o -> t k o", k=4)
                    nc.gpsimd.dma_start(
                        out=dst, in_=of[:].rearrange("p (k o) -> p k o", k=4))

                for k in range(NB + LAG):
                    if k < NB:
                        if k == 0:
                            u_mms(0)
                            emit_masks()
                            u_evict(0)
                        else:
                            half_iter(k, 0)
                    if k >= LAG:
                        b = k - LAG
                        if b == 0:
                            u_mms(1)
                            u_evict(1)
                        else:
                            half_iter(b, 1)

            if debug:
                nc.sync.dma_start(out=xt_dump[:, :], in_=xT2[:])
                nc.sync.dma_start(out=um_dump[:, :], in_=u_m[:])
                nc.sync.dma_start(out=mask_dump[:, :], in_=maskT[:])
    nc.finalize()
    return nc


_NC = None


def _get_nc():
    global _NC
    if _NC is None:
        _NC = _build()
    return _NC


class _Runner:
    """Cached PJRT executable for the SPMD bass kernel."""

    _CORE_SHARDED = {"xt", "tl"}

    def __init__(self):
        import jax
        import concourse.mybir as mybir_
        from concourse import bass2jax

        bass2jax.install_neuronx_cc_hook()
        self._bass2jax = bass2jax
        nc = _get_nc()
        self.nc = nc

        partition_name = (nc.partition_id_tensor.name
                          if nc.partition_id_tensor else None)
        in_names, out_names, out_avals, zero_outs = [], [], [], []
        for alloc in nc.m.functions[0].allocations:
            if not isinstance(alloc, mybir_.MemoryLocationSet):
                continue
            name = alloc.memorylocations[0].name
            if alloc.kind == "ExternalInput":
                if name != partition_name:
                    in_names.append(name)
            elif alloc.kind == "ExternalOutput":
                shape = tuple(alloc.tensor_shape)
                dtype = mybir_.dt.np(alloc.dtype)
                out_names.append(name)
                out_avals.append(jax.core.ShapedArray(shape, dtype))
                zero_outs.append((shape, dtype))
        self.in_names = list(in_names)
        self.out_names = out_names
        self.out_avals = out_avals
        all_in_names = in_names + out_names
        if partition_name is not None:
            all_in_names.append(partition_name)

        from jax.experimental.shard_map import shard_map
        from jax.sharding import Mesh, NamedSharding, PartitionSpec

        devices = jax.devices()[:NCORES]
        assert len(devices) == NCORES, devices
        mesh = Mesh(np.asarray(devices), ("core",))
        self.mesh = mesh

        def spec_for(name):
            return (PartitionSpec("core") if name in self._CORE_SHARDED
                    else PartitionSpec())

        in_specs = tuple(spec_for(n) for n in in_names) + \
            (PartitionSpec("core"),) * len(out_names)
        out_specs = (PartitionSpec("core"),) * len(out_names)
        self.in_shardings = [NamedSharding(mesh, spec_for(n))
                             for n in in_names]
        self.out_sharding = NamedSharding(mesh, PartitionSpec("core"))

        def _body(*args):
            operands = list(args)
            if partition_name is not None:
                operands.append(bass2jax.partition_id_tensor())
            outs = bass2jax._bass_exec_p.bind(
                *operands,
                out_avals=tuple(out_avals),
                in_names=tuple(all_in_names),
                out_names=tuple(out_names),
                lowering_input_output_aliases=(),
                sim_require_finite=True,
                sim_require_nnan=True,
                nc=nc,
            )
            return tuple(outs)

        self._fn = jax.jit(
            shard_map(_body, mesh=mesh, in_specs=in_specs,
                      out_specs=out_specs, check_rep=False),
            keep_unused=True)
        self._scratch_dev = [
            jax.device_put(
                np.zeros((NCORES * a.shape[0], *a.shape[1:]), a.dtype),
                self.out_sharding)
            for a in out_avals
        ]

    def put_inputs(self, by_name):
        import jax
        out = []
        for name, sharding in zip(self.in_names, self.in_shardings):
            out.append(jax.device_put(by_name[name], sharding))
        return out

    def run_device(self, dev_args):
        return self._fn(*dev_args, *self._scratch_dev)

    def run(self, by_name):
        outs = self.run_device(self.put_inputs(by_name))
        host = [np.asarray(o) for o in outs]
        return {n: h for n, h in zip(self.out_names, host)}


_RUNNER = None


def _get_runner():
    global _RUNNER
    if _RUNNER is None:
        _RUNNER = _Runner()
    return _RUNNER


def _global_inputs(x, W_base, b_base, WA, WB, scaling, token_lora):
    """Full-size (global) arrays keyed by DRAM-parameter name.

    Weight tensors are prepped on host (cast to bf16 + blocked
    transposed layouts); activations (x, token_lora) pass through
    untouched.
    """
    import ml_dtypes
    BF = ml_dtypes.bfloat16

    W = np.asarray(W_base, dtype=np.float32)
    wa_flat = np.asarray(WA, dtype=np.float32).reshape(LR, D)
    wtb = np.empty((NB, P, ND * P), dtype=BF)
    for b in range(NB):
        src = wa_flat if b == 0 else W[(b - 1) * P:b * P]
        blk = src.T.reshape(ND, P, P).transpose(1, 0, 2).reshape(P, ND * P)
        wtb[b] = blk.astype(BF)
    wbs = (np.asarray(WB, dtype=np.float32)
           * np.asarray(scaling, dtype=np.float32)[:, None, None])
    wbst = np.ascontiguousarray(
        wbs.transpose(0, 2, 1).reshape(LR, O)).astype(BF)
    bbv = np.asarray(b_base, dtype=np.float32).reshape(NOC, P).T
    jdiv = (np.arange(P, dtype=np.float32) // 16.0).reshape(P, 1)
    cvec = np.ascontiguousarray(np.concatenate([jdiv, bbv], axis=1))
    # x^T in the exact per-core xT2 SBUF image: rows [c*128:(c+1)*128]
    # hold core c's [p, (h, dc, t)] layout, (p,h,dc,t) = x[c*1024 +
    # h*512 + t, dc*128 + p], bf16.
    xbf = np.asarray(x, dtype=np.float32).astype(BF)
    xtg = np.ascontiguousarray(
        xbf.reshape(NCORES, 2, HT, ND, P).transpose(0, 4, 1, 3, 2)
        .reshape(NCORES * P, NT * ND * P))
    return {
        "xt": xtg,
        "tl": np.ascontiguousarray(np.asarray(token_lora, dtype=np.int32)),
        "wtb": np.ascontiguousarray(wtb),
        "wbst": wbst,
        "cvec": cvec,
    }


def kernel(x, W_base, b_base, WA, WB, scaling, token_lora):
    by_name = _global_inputs(x, W_base, b_base, WA, WB, scaling, token_lora)
    try:
        res = _get_runner().run(by_name)
        return res["out"]
    except Exception:
        # robust fallback through the library SPMD path
        from concourse.bass_utils import run_bass_kernel_spmd

        nc = _get_nc()
        in_maps = []
        for c in range(NCORES):
            in_maps.append({
                "xt": by_name["xt"][c * P:(c + 1) * P],
                "tl": by_name["tl"][c * TS:(c + 1) * TS],
                "wtb": by_name["wtb"],
                "wbst": by_name["wbst"],
                "cvec": by_name["cvec"],
            })
        res = run_bass_kernel_spmd(nc, in_maps, core_ids=list(range(NCORES)))
        return np.concatenate(
            [res.results[c]["out"] for c in range(NCORES)], axis=0)


# revision 43
# speedup vs baseline: 1.0007x; 1.0007x over previous
"""Trainium2 Bass kernel for nn_MixedLoraModel_734 (v2).

Computes, for T=8192 tokens, D=O=4096:
    out = x @ W_base^T + b_base + scaling[token_lora][:,None] * lora(x)
where lora(x)[t] = WB[l_t] @ (WA[l_t] @ x[t]),  l_t = token_lora[t],
L=8 adapters of rank R=16 (adapter stack = 128 rows).

Strategy (8 NeuronCores, data-parallel over tokens):
  - Inputs are prepped on host (bf16 casts + blocked transposed
    layouts, as a serving stack would do offline for weights):
    W_base^T in oc-blocked layout wtb[33, 128, 32*128] (block 0 =
    WA_flat^T for the LoRA `u` GEMM, blocks 1..32 = 128-wide o-chunks
    of W^T), wbsT = bf16 (scaling * WB)^T in [(l,r), o] layout, bias +
    jdiv packed into cvec[128, 33], and x^T delivered in the exact
    per-core xT2 SBUF image ([p, (h, dc, t)], so every moving operand
    is a plain contiguous 512-wide slab).
  - On device the PE does *only* matmuls at peak bf16 cadence (216 ns
    per 512-wide matmul): per o-chunk, 32 stationary [d,o] tiles
    stream from DRAM against the resident bf16 x^T; one extra matmul
    per chunk adds the routed LoRA term; the u-GEMM is just block 0 of
    the same pipeline.
  - Startup rides 3 load channels (sync/scalar HWDGE + gpsimd SWDGE,
    each ~95-150 GB/s, one DMA at a time), h0-critical bytes first;
    token halves are software-pipelined (h0 runs LAG blocks ahead of
    h1) so the PE starts before the second half of x has landed.
  - Routing is dense: maskT[j, t] = (token_lora[t] == j//16) built
    from a PE broadcast + DVE compare (emitted after the first block's
    matmuls to avoid head-blocking the in-order PE stream);
    u_m = mask * u evicted to bf16.
  - Output path: scalar eviction (bias add, bf16 cast) -> DMA-XBAR
    transpose back to natural [t, o] (sync queue only: two concurrent
    XBAR transposes on different HW queues corrupt each other) -> DVE
    cast to f32 -> store via the gpsimd software-DGE queue.
"""

import numpy as np

import concourse.bass as bass
import concourse.mybir as mybir
import concourse.tile as tile
from concourse import bacc

P = 128
D = 4096          # d_in
O = 4096          # d_out
NCORES = 8
T = 8192
TS = T // NCORES  # 1024 tokens per core
NT = TS // P      # 8 token tiles per core
ND = D // P       # 32 contraction chunks
NOC = O // P      # 32 output chunks of 128
HT = TS // 2      # 512-wide halves
NB = NOC + 1      # u block + 32 o-chunks
L, R, LR = 8, 16, 128
LAG = 4           # h0 runs LAG blocks ahead of h1

F32 = mybir.dt.float32
BF16 = mybir.dt.bfloat16
I32 = mybir.dt.int32
EQ = mybir.AluOpType.is_equal
MUL = mybir.AluOpType.mult
IDENT = mybir.ActivationFunctionType.Identity


def _build(debug: bool = False) -> bass.Bass:
    nc = bacc.Bacc(None)

    xt = nc.declare_dram_parameter("xt", [P, NT * ND * P], BF16,
                                   isOutput=False)
    tl = nc.declare_dram_parameter("tl", [TS], I32, isOutput=False)
    wtb = nc.declare_dram_parameter("wtb", [NB, P, ND * P], BF16,
                                    isOutput=False)
    wbst = nc.declare_dram_parameter("wbst", [P, O], BF16, isOutput=False)
    cvec = nc.declare_dram_parameter("cvec", [P, 1 + NOC], F32,
                                     isOutput=False)
    out = nc.declare_dram_parameter("out", [TS, O], F32, isOutput=True)
    if debug:
        xt_dump = nc.declare_dram_parameter("xt_dump", [P, NT * ND * P],
                                            BF16, isOutput=True)
        um_dump = nc.declare_dram_parameter("um_dump", [P, TS], BF16,
                                            isOutput=True)
        mask_dump = nc.declare_dram_parameter("mask_dump", [P, TS], F32,
                                              isOutput=True)

    with tile.TileContext(nc) as tc:
        with (
            tc.tile_pool(name="const", bufs=1) as const,
            tc.tile_pool(name="res", bufs=1) as res,
            tc.tile_pool(name="wt", bufs=7) as wt_p,
            tc.tile_pool(name="outT", bufs=3) as outT_p,
            tc.tile_pool(name="onat", bufs=3) as onat_p,
            tc.tile_pool(name="outf", bufs=3) as outf_p,
        ):
            # ---- resident tiles ----
            # xT2 layout: [p, (h, dc, t)] — x^T in half-token-major,
            # contraction-chunk blocks; each (h, dc) slab is a contiguous
            # 512-wide moving operand.
            xT2 = res.tile([P, NT * ND * P], BF16, tag="xT2")
            wbsT = res.tile([P, O], BF16, tag="wbsT")
            cvec_sb = const.tile([P, 1 + NOC], F32)
            jdiv_sb = cvec_sb[:, 0:1]
            u_m = res.tile([P, TS], BF16, tag="u_m")
            maskT = res.tile([P, TS], F32, tag="maskT")
            ones_row = const.tile([1, P], F32)
            tli = const.tile([1, TS], I32)
            tlf = const.tile([1, TS], F32)

            # ---- emission helpers ----
            wt_tiles = {}

            def w_load(b, q):
                wt = wt_p.tile([P, ND * P], BF16, tag="wt", name=f"wt{b}")
                q.dma_start(out=wt[:], in_=wtb[b, :, :])
                wt_tiles[b] = wt

            # ---- prepass emission ----
            # x^T arrives from the host already in the xT2 SBUF image, so
            # ingestion is 4 plain contiguous DMAs split across both HW
            # queues (quarter-token-halves; the tile scheduler chains big
            # DMAs with false serialization edges, so keep them few and
            # balanced).  XBAR transposes are reserved for the out path
            # (sync queue only: two concurrent XBAR transposes on
            # different queues corrupt each other).
            # Three load channels (sync HWDGE, scalar HWDGE, gpsimd
            # SWDGE), h0-critical bytes first on each.  Per-queue DMAs
            # run one-at-a-time (~95-150 GB/s each), so balance by bytes.
            XCW = NT * ND * P // 8  # x chunk width (8 chunks, asc = h0 1st)

            def x_chunk(k, q):
                q.dma_start(out=xT2[:, k * XCW:(k + 1) * XCW],
                            in_=xt[:, k * XCW:(k + 1) * XCW])

            # W0 split in halves so the very first u matmuls unblock early
            wt0 = wt_p.tile([P, ND * P], BF16, tag="wt", name="wt0")
            wt_tiles[0] = wt0
            HW = ND * P // 2
            nc.gpsimd.dma_start(out=wt0[:, 0:HW], in_=wtb[0, :, 0:HW])
            nc.gpsimd.dma_start(out=wt0[:, HW:2 * HW],
                                in_=wtb[0, :, HW:2 * HW])
            x_chunk(0, nc.sync)
            x_chunk(1, nc.scalar)
            nc.gpsimd.dma_start(out=tli[:],
                                in_=tl.rearrange("(a f) -> a f", a=1))
            nc.gpsimd.dma_start(out=cvec_sb[:], in_=cvec[:, :])
            x_chunk(2, nc.gpsimd)
            x_chunk(3, nc.sync)
            x_chunk(4, nc.scalar)
            w_load(1, nc.gpsimd)
            x_chunk(6, nc.sync)
            x_chunk(7, nc.scalar)
            nc.gpsimd.dma_start(out=wbsT[:], in_=wbst[:, :])
            x_chunk(5, nc.gpsimd)
            w_load(2, nc.sync)
            w_load(3, nc.scalar)
            w_load(4, nc.gpsimd)
            nc.vector.memset(ones_row[:], 1.0)
            nc.vector.tensor_copy(tlf[:], tli[:])

            def emit_masks():
                # routing mask: broadcast token ids across partitions
                # (PE), compare against j//16 (DVE).  Emitted AFTER the
                # first half-iter's matmuls so the in-order PE stream is
                # not head-blocked waiting for the token-id row.
                with tc.tile_pool(name="psM", bufs=2, space="PSUM") as psM:
                    for h in range(2):
                        tlbc = psM.tile([P, HT], F32, tag="tlbc",
                                        name=f"tlbc{h}")
                        nc.tensor.matmul(tlbc[:], ones_row[:],
                                         tlf[0:1, h * HT:(h + 1) * HT],
                                         start=True, stop=True)
                        nc.vector.tensor_scalar(
                            maskT[:, h * HT:(h + 1) * HT],
                            tlbc[:], jdiv_sb, None, EQ)

            # ---- main loop: software-pipelined halves ----
            with (
                tc.tile_pool(name="psU", bufs=2, space="PSUM") as psU,
                tc.tile_pool(name="psA", bufs=4, space="PSUM") as psA,
            ):
                u_ps = {}
                next_w = 5

                def u_mms(h):
                    acc = psU.tile([P, HT], F32, tag="ups", name=f"ups{h}")
                    u_ps[h] = acc
                    wt = wt_tiles[0]
                    for dc in range(ND):
                        mov = xT2[:, (h * ND + dc) * HT:
                                  (h * ND + dc + 1) * HT]
                        nc.tensor.matmul(acc[:],
                                         wt[:, dc * P:(dc + 1) * P], mov,
                                         start=(dc == 0),
                                         stop=(dc == ND - 1))

                def u_evict(h):
                    # mask + bf16 eviction of the selected u
                    nc.vector.tensor_tensor(
                        u_m[:, h * HT:(h + 1) * HT], u_ps[h][:],
                        maskT[:, h * HT:(h + 1) * HT], MUL)

                def half_iter(b, h):
                    nonlocal next_w
                    if h == 0 and next_w < NB:
                        w_load(next_w, nc.sync if next_w % 2 else nc.scalar)
                        next_w += 1
                    wt = wt_tiles[b]
                    acc = psA.tile([P, HT], F32, tag="acc",
                                   name=f"acc{b}_{h}")
                    for dc in range(ND):
                        mov = xT2[:, (h * ND + dc) * HT:
                                  (h * ND + dc + 1) * HT]
                        nc.tensor.matmul(acc[:],
                                         wt[:, dc * P:(dc + 1) * P], mov,
                                         start=(dc == 0), stop=False)
                    oc = b - 1
                    nc.tensor.matmul(acc[:], wbsT[:, oc * P:(oc + 1) * P],
                                     u_m[:, h * HT:(h + 1) * HT],
                                     start=False, stop=True)
                    # evict: bias add + bf16 cast on scalar engine
                    oT = outT_p.tile([P, HT], BF16, tag="oT",
                                     name=f"oT{oc}_{h}")
                    nc.scalar.activation(oT[:], acc[:], IDENT,
                                         bias=cvec_sb[:, 1 + oc:2 + oc])
                    # XBAR transpose back to natural token-major layout
                    onat = onat_p.tile([P, HT], BF16, tag="onat",
                                       name=f"onat{oc}_{h}")
                    # sync queue only: XBAR transposes must never overlap
                    nc.sync.dma_start_transpose(
                        onat[:].rearrange("p (k o) -> p k o", o=P), oT[:])
                    # f32 cast + store via gpsimd software DGE
                    of = outf_p.tile([P, HT], F32, tag="of",
                                     name=f"of{oc}_{h}")
                    nc.vector.tensor_copy(of[:], onat[:])
                    dst = out[h * HT:(h + 1) * HT, oc * P:(oc + 1) * P] \
                        .rearrange("(k t) o -> t k o", k=4)
                    nc.gpsimd.dma_start(
                        out=dst, in_=of[:].rearrange("p (k o) -> p k o", k=4))

                for k in range(NB + LAG):
                    if k < NB:
                        if k == 0:
                            u_mms(0)
                            emit_masks()
                            u_evict(0)
                        else:
                            half_iter(k, 0)
                    if k >= LAG:
                        b = k - LAG
                        if b == 0:
                            u_mms(1)
                            u_evict(1)
                        else:
                            half_iter(b, 1)

            if debug:
                nc.sync.dma_start(out=xt_dump[:, :], in_=xT2[:])
                nc.sync.dma_start(out=um_dump[:, :], in_=u_m[:])
                nc.sync.dma_start(out=mask_dump[:, :], in_=maskT[:])
    nc.finalize()
    return nc


_NC = None


def _get_nc():
    global _NC
    if _NC is None:
        _NC = _build()
    return _NC


class _Runner:
    """Cached PJRT executable for the SPMD bass kernel."""

    _CORE_SHARDED = {"xt", "tl"}

    def __init__(self):
        import jax
        import concourse.mybir as mybir_
        from concourse import bass2jax

        bass2jax.install_neuronx_cc_hook()
        self._bass2jax = bass2jax
        nc = _get_nc()
        self.nc = nc

        partition_name = (nc.partition_id_tensor.name
                          if nc.partition_id_tensor else None)
        in_names, out_names, out_avals, zero_outs = [], [], [], []
        for alloc in nc.m.functions[0].allocations:
            if not isinstance(alloc, mybir_.MemoryLocationSet):
                continue
            name = alloc.memorylocations[0].name
            if alloc.kind == "ExternalInput":
                if name != partition_name:
                    in_names.append(name)
            elif alloc.kind == "ExternalOutput":
                shape = tuple(alloc.tensor_shape)
                dtype = mybir_.dt.np(alloc.dtype)
                out_names.append(name)
                out_avals.append(jax.core.ShapedArray(shape, dtype))
                zero_outs.append((shape, dtype))
        self.in_names = list(in_names)
        self.out_names = out_names
        self.out_avals = out_avals
        all_in_names = in_names + out_names
        if partition_name is not None:
            all_in_names.append(partition_name)

        from jax.experimental.shard_map import shard_map
        from jax.sharding import Mesh, NamedSharding, PartitionSpec

        devices = jax.devices()[:NCORES]
        assert len(devices) == NCORES, devices
        mesh = Mesh(np.asarray(devices), ("core",))
        self.mesh = mesh

        def spec_for(name):
            return (PartitionSpec("core") if name in self._CORE_SHARDED
                    else PartitionSpec())

        in_specs = tuple(spec_for(n) for n in in_names) + \
            (PartitionSpec("core"),) * len(out_names)
        out_specs = (PartitionSpec("core"),) * len(out_names)
        self.in_shardings = [NamedSharding(mesh, spec_for(n))
                             for n in in_names]
        self.out_sharding = NamedSharding(mesh, PartitionSpec("core"))

        def _body(*args):
            operands = list(args)
            if partition_name is not None:
                operands.append(bass2jax.partition_id_tensor())
            outs = bass2jax._bass_exec_p.bind(
                *operands,
                out_avals=tuple(out_avals),
                in_names=tuple(all_in_names),
                out_names=tuple(out_names),
                lowering_input_output_aliases=(),
                sim_require_finite=True,
                sim_require_nnan=True,
                nc=nc,
            )
            return tuple(outs)

        self._fn = jax.jit(
            shard_map(_body, mesh=mesh, in_specs=in_specs,
                      out_specs=out_specs, check_rep=False),
            keep_unused=True)
        self._scratch_dev = [
            jax.device_put(
                np.zeros((NCORES * a.shape[0], *a.shape[1:]), a.dtype),
                self.out_sharding)
            for a in out_avals
        ]

    def put_inputs(self, by_name):
        import jax
        out = []
        for name, sharding in zip(self.in_names, self.in_shardings):
            out.append(jax.device_put(by_name[name], sharding))
        return out

    def run_device(self, dev_args):
        return self._fn(*dev_args, *self._scratch_dev)

    def run(self, by_name):
        outs = self.run_device(self.put_inputs(by_name))
        host = [np.asarray(o) for o in outs]
        return {n: h for n, h in zip(self.out_names, host)}


_RUNNER = None


def _get_runner():
    global _RUNNER
    if _RUNNER is None:
        _RUNNER = _Runner()
    return _RUNNER


def _global_inputs(x, W_base, b_base, WA, WB, scaling, token_lora):
    """Full-size (global) arrays keyed by DRAM-parameter name.

    Weight tensors are prepped on host (cast to bf16 + blocked
    transposed layouts); activations (x, token_lora) pass through
    untouched.
    """
    import ml_dtypes
    BF = ml_dtypes.bfloat16

    W = np.asarray(W_base, dtype=np.float32)
    wa_flat = np.asarray(WA, dtype=np.float32).reshape(LR, D)
    wtb = np.empty((NB, P, ND * P), dtype=BF)
    for b in range(NB):
        src = wa_flat if b == 0 else W[(b - 1) * P:b * P]
        blk = src.T.reshape(ND, P, P).transpose(1, 0, 2).reshape(P, ND * P)
        wtb[b] = blk.astype(BF)
    wbs = (np.asarray(WB, dtype=np.float32)
           * np.asarray(scaling, dtype=np.float32)[:, None, None])
    wbst = np.ascontiguousarray(
        wbs.transpose(0, 2, 1).reshape(LR, O)).astype(BF)
    bbv = np.asarray(b_base, dtype=np.float32).reshape(NOC, P).T
    jdiv = (np.arange(P, dtype=np.float32) // 16.0).reshape(P, 1)
    cvec = np.ascontiguousarray(np.concatenate([jdiv, bbv], axis=1))
    # x^T in the exact per-core xT2 SBUF image: rows [c*128:(c+1)*128]
    # hold core c's [p, (h, dc, t)] layout, (p,h,dc,t) = x[c*1024 +
    # h*512 + t, dc*128 + p], bf16.
    xbf = np.asarray(x, dtype=np.float32).astype(BF)
    xtg = np.ascontiguousarray(
        xbf.reshape(NCORES, 2, HT, ND, P).transpose(0, 4, 1, 3, 2)
        .reshape(NCORES * P, NT * ND * P))
    return {
        "xt": xtg,
        "tl": np.ascontiguousarray(np.asarray(token_lora, dtype=np.int32)),
        "wtb": np.ascontiguousarray(wtb),
        "wbst": wbst,
        "cvec": cvec,
    }


def kernel(x, W_base, b_base, WA, WB, scaling, token_lora):
    by_name = _global_inputs(x, W_base, b_base, WA, WB, scaling, token_lora)
    try:
        res = _get_runner().run(by_name)
        return res["out"]
    except Exception:
        # robust fallback through the library SPMD path
        from concourse.bass_utils import run_bass_kernel_spmd

        nc = _get_nc()
        in_maps = []
        for c in range(NCORES):
            in_maps.append({
                "xt": by_name["xt"][c * P:(c + 1) * P],
                "tl": by_name["tl"][c * TS:(c + 1) * TS],
                "wtb": by_name["wtb"],
                "wbst": by_name["wbst"],
                "cvec": by_name["cvec"],
            })
        res = run_bass_kernel_spmd(nc, in_maps, core_ids=list(range(NCORES)))
        return np.concatenate(
            [res.results[c]["out"] for c in range(NCORES)], axis=0)
